# revision 11
# baseline (speedup 1.0000x reference)
"""MoE block (8 experts, top-2, + shared expert) on 8 trn2 NeuronCores.

Strategy (expert-parallel, host dispatch):
  - Host computes gate logits/softmax/top-2 (0.03% of total FLOPs) and
    dispatches tokens: core c receives the tokens routed to expert c
    (padded to the max per-expert count) plus a 1/8 slice of all tokens
    for the shared expert.
  - Each core runs two FFN passes in one Bass program: shared FFN on its
    512-token slice, then expert-c FFN on its routed tokens. Matmuls are
    bf16 (weights + activations) with fp32 PSUM accumulation; feature-major
    ([D, tokens]) layout avoids all on-device transposes.
  - Host combines: routed outputs scaled by renormalized top-2 weights and
    scatter-added, shared outputs added per-slice, biases b2/bs2 added on
    host (they enter linearly).

Layout per core (SPMD, same program all 8 cores):
  inputs : xt [1024, NT] bf16  (cols = [512 shared | C routed])
           w1e [1024,4096] bf16, w2e [4096,1024] bf16  (expert c)
           ws1 [1024,4096] bf16, ws2 [4096,1024] bf16  (shared, replicated)
           b1r/bs1r [128,32] fp32 (b1 reshaped (f=ft*128+p) -> [p, ft])
  output : yt [1024, NT] fp32
"""

import numpy as np
import ml_dtypes

import concourse.bass as bass
import concourse.bacc as bacc
from concourse import mybir
from concourse.tile import TileContext, add_dep_helper
from concourse.bass_utils import run_bass_kernel_spmd

D = 1024
FF = 4096
E = 8
TOPK = 2
B, L = 4, 1024
T = B * L
NCORES = 8
SHARED = T // NCORES  # shared-expert tokens per core
P = 128
DT = D // P    # 8 k-tiles for D
FT = FF // P   # 32 tiles for FF

_BF16 = mybir.dt.bfloat16
_F32 = mybir.dt.float32

_program_cache: dict[tuple, tuple] = {}

# test harness hooks: extra kwargs for run_bass_kernel_spmd (e.g. trace=True)
# and the last BassKernelResults for profiling. Unused in normal grading runs.
TRACE_KWARGS: dict = {}
last_results = None


def _chunk_plan(C: int) -> list[int]:
    """Split C routed columns into <=512-wide chunks, sizes multiple of 8."""
    n = -(-C // 512)
    base = -(-C // n)
    base = -(-base // 8) * 8
    sizes = []
    left = C
    for _ in range(n - 1):
        sizes.append(base)
        left -= base
    sizes.append(left)
    assert all(0 < s <= 512 for s in sizes) and sum(sizes) == C
    return sizes


def _build_program(C: int):
    """One SPMD Bass program: shared FFN (512 cols) + expert FFN (C cols)."""
    NT = SHARED + C
    nc = bacc.Bacc()

    xt = nc.dram_tensor("xt", [D, NT], _BF16, kind="ExternalInput")
    w1e = nc.dram_tensor("w1e", [D, FF], _BF16, kind="ExternalInput")
    w2e = nc.dram_tensor("w2e", [FF, D], _BF16, kind="ExternalInput")
    ws1 = nc.dram_tensor("ws1", [D, FF], _BF16, kind="ExternalInput")
    ws2 = nc.dram_tensor("ws2", [FF, D], _BF16, kind="ExternalInput")
    b1r = nc.dram_tensor("b1r", [P, FT], _F32, kind="ExternalInput")
    bs1r = nc.dram_tensor("bs1r", [P, FT], _F32, kind="ExternalInput")
    yt = nc.dram_tensor("yt", [D, NT], _F32, kind="ExternalOutput")

    # chunks: (weights_key, col_offset, width); shared phase first so the
    # expert weights can stream in (reusing the same SBUF slots) while the
    # shared phase computes.
    chunks = [("s", 0, SHARED)]
    off = SHARED
    for w in _chunk_plan(C):
        chunks.append(("e", off, w))
        off += w

    with TileContext(nc) as tc:
        with (
            tc.tile_pool(name="wpool", bufs=1) as wpool,
            tc.tile_pool(name="xpool", bufs=1) as xpool,
            tc.tile_pool(name="hpool", bufs=34) as hpool,
            tc.tile_pool(name="ypool", bufs=8) as ypool,
            tc.tile_pool(name="bpool", bufs=1) as bpool,
            tc.tile_pool(name="psum", bufs=4, space="PSUM") as psum,
        ):
            b1t = bpool.tile([P, FT], _F32, tag="b1", name="b1t")
            nc.sync.dma_start(b1t, b1r[:, :])
            bs1t = bpool.tile([P, FT], _F32, tag="bs1", name="bs1t")
            nc.sync.dma_start(bs1t, bs1r[:, :])

            # W1 is stored as 8x8 [128 (d), 512 (f)] tiles (8 f-groups) so
            # the first matmul group only waits for ~1MB of DMA, not the
            # whole 8MB tensor; g-major issue order puts group 0 at the
            # DMA-queue fronts.
            FG = 8
            FGW = FF // FG  # 512 weight columns per f-group

            # The first matmul group only needs Ws1 f-group 0 + chunk-0 X
            # (~1.8MB). Bulk weight DMAs are held behind the last critical
            # DMA so the critical path gets full HBM bandwidth.
            hold_after = None
            last_dma = [None]

            def _dma(dst, src):
                i = nc.sync.dma_start(dst, src)
                last_dma[0] = i
                if hold_after is not None:
                    add_dep_helper(i.ins, hold_after,
                                   reason="bulk load after critical prefetch")
                return i

            def load_w1(src1, pfx, groups):
                t1 = {}
                for g in groups:
                    for d in range(DT):
                        t = wpool.tile([P, FGW], _BF16, tag=f"w1_{d}_{g}",
                                       name=f"{pfx}w1_{d}_{g}")
                        _dma(t, src1[d * P:(d + 1) * P, g * FGW:(g + 1) * FGW])
                        t1[(d, g)] = t
                return t1

            def load_w2(src2, pfx):
                t2 = []
                for f in range(FT):
                    t = wpool.tile([P, D], _BF16, tag=f"w2_{f}", name=f"{pfx}w2_{f}")
                    _dma(t, src2[f * P:(f + 1) * P, :])
                    t2.append(t)
                return t2

            def load_x(off, N, pfx):
                xts = []
                for d in range(DT):
                    t = xpool.tile([P, 512], _BF16, tag=f"x_{d}", name=f"{pfx}x_{d}")
                    t = t[:, :N]
                    _dma(t, xt[d * P:(d + 1) * P, off:off + N])
                    xts.append(t)
                return xts

            # DMA issue order: Ws1 group 0 + chunk-0 X (critical), then bulk.
            ws1_t = load_w1(ws1, "s_", [0])
            x0 = load_x(chunks[0][1], chunks[0][2], "c0_")
            hold_after = last_dma[0].ins
            ws1_t.update(load_w1(ws1, "s_", range(1, FG)))
            ws2_t = load_w2(ws2, "s_")
            we1_t = we2_t = None

            for ci, (kind, off, N) in enumerate(chunks):
                if kind == "s":
                    w1t, w2t, bt = ws1_t, ws2_t, bs1t
                else:
                    if we1_t is None:
                        we1_t = load_w1(w1e, "e_", range(FG))
                        we2_t = load_w2(w2e, "e_")
                    w1t, w2t, bt = we1_t, we2_t, b1t

                xts = x0 if ci == 0 else load_x(off, N, f"c{ci}_")

                hts = []
                for f in range(FT):
                    ph = psum.tile([P, 512], _F32, tag="ph", name="ph")[:, :N]
                    g, fi = divmod(f, FT // FG)
                    for d in range(DT):
                        nc.tensor.matmul(
                            ph,
                            lhsT=w1t[(d, g)][:, fi * P:(fi + 1) * P],
                            rhs=xts[d],
                            start=(d == 0),
                            stop=(d == DT - 1),
                        )
                    ht = hpool.tile([P, 512], _BF16, tag="h", name="h")[:, :N]
                    nc.scalar.activation(
                        ht, ph, mybir.ActivationFunctionType.Gelu,
                        bias=bt[:, f:f + 1],
                    )
                    hts.append(ht)

                for d in range(DT):
                    py = psum.tile([P, 512], _F32, tag="py", name="py")[:, :N]
                    for f in range(FT):
                        nc.tensor.matmul(
                            py,
                            lhsT=w2t[f][:, d * P:(d + 1) * P],
                            rhs=hts[f],
                            start=(f == 0),
                            stop=(f == FT - 1),
                        )
                    yo = ypool.tile([P, 512], _F32, tag="y", name="y")[:, :N]
                    nc.vector.tensor_copy(yo, py)
                    nc.sync.dma_start(yt[d * P:(d + 1) * P, off:off + N], yo)

    nc.finalize()
    return nc


def _get_program(C: int):
    key = (C,)
    if key not in _program_cache:
        _program_cache[key] = _build_program(C)
    return _program_cache[key]


def _route(xf: np.ndarray, W_gate: np.ndarray):
    """Replicate the reference gate in float64 (selection margins are ~1e-5,
    far above fp32 rounding, so the top-2 sets match the fp32 reference)."""
    logits = xf.astype(np.float64) @ W_gate.astype(np.float64)
    m = logits.max(axis=-1, keepdims=True)
    p = np.exp(logits - m)
    p /= p.sum(axis=-1, keepdims=True)
    top_i = np.argsort(-p, axis=-1, kind="stable")[:, :TOPK]
    top_v = np.take_along_axis(p, top_i, axis=-1)
    top_v = top_v / top_v.sum(axis=-1, keepdims=True)
    return top_i, top_v.astype(np.float32)


def kernel(x, W_gate, W1, b1, W2, b2, Ws1, bs1, Ws2, bs2):
    x = np.asarray(x, np.float32)
    xf = x.reshape(T, D)
    top_i, top_v = _route(xf, np.asarray(W_gate, np.float32))

    # per-expert token lists
    idx = [np.nonzero((top_i == e).any(axis=1))[0] for e in range(E)]
    wgt = []
    for e in range(E):
        sel = top_i[idx[e]] == e  # [cnt, K] exactly one True per row
        wgt.append(top_v[idx[e]][sel].astype(np.float32))
    counts = np.array([len(i) for i in idx])
    C = int(-(-counts.max() // 8) * 8)
    NT = SHARED + C

    xbf = xf.astype(ml_dtypes.bfloat16)
    W1 = np.asarray(W1); W2 = np.asarray(W2)
    ws1_b = np.ascontiguousarray(np.asarray(Ws1, np.float32).astype(ml_dtypes.bfloat16))
    ws2_b = np.ascontiguousarray(np.asarray(Ws2, np.float32).astype(ml_dtypes.bfloat16))
    bs1r = np.ascontiguousarray(
        np.asarray(bs1, np.float32).reshape(FT, P).T)

    in_maps = []
    for c in range(E):
        pad_idx = np.zeros(C, np.int64)
        pad_idx[:counts[c]] = idx[c]
        xcols = np.concatenate([xbf[c * SHARED:(c + 1) * SHARED], xbf[pad_idx]], axis=0)
        in_maps.append({
            "xt": np.ascontiguousarray(xcols.T),
            "w1e": np.ascontiguousarray(np.asarray(W1[c], np.float32).astype(ml_dtypes.bfloat16)),
            "w2e": np.ascontiguousarray(np.asarray(W2[c], np.float32).astype(ml_dtypes.bfloat16)),
            "ws1": ws1_b,
            "ws2": ws2_b,
            "b1r": np.ascontiguousarray(np.asarray(b1[c], np.float32).reshape(FT, P).T),
            "bs1r": bs1r,
        })

    nc = _get_program(C)
    global last_results
    last_results = run_bass_kernel_spmd(
        nc, in_maps, list(range(NCORES)), **TRACE_KWARGS)
    res = last_results.results

    out = np.zeros((T, D), np.float32)
    for c in range(E):
        y = np.asarray(res[c]["yt"], np.float32)
        out[c * SHARED:(c + 1) * SHARED] += y[:, :SHARED].T
        cnt = counts[c]
        out[idx[c]] += wgt[c][:, None] * y[:, SHARED:SHARED + cnt].T

    # biases enter linearly; add on host (zeros in this problem's inputs)
    b2 = np.asarray(b2, np.float32)
    bs2 = np.asarray(bs2, np.float32)
    combine = np.zeros((T, E), np.float32)
    np.put_along_axis(combine, top_i, top_v, axis=1)
    out += combine @ b2 + bs2

    return out.reshape(B, L, D)


# revision 14
# speedup vs baseline: 1.0107x; 1.0107x over previous
"""MoE block (8 experts, top-2, + shared expert) on 8 trn2 NeuronCores.

Strategy (expert-parallel, host dispatch):
  - Host computes gate logits/softmax/top-2 (0.03% of total FLOPs) and
    dispatches tokens: core c receives the tokens routed to expert c
    (padded to the max per-expert count) plus a 1/8 slice of all tokens
    for the shared expert.
  - Each core runs two FFN passes in one Bass program: shared FFN on its
    512-token slice, then expert-c FFN on its routed tokens. Matmuls are
    bf16 (weights + activations) with fp32 PSUM accumulation; feature-major
    ([D, tokens]) layout avoids all on-device transposes.
  - Host combines: routed outputs scaled by renormalized top-2 weights and
    scatter-added, shared outputs added per-slice, biases b2/bs2 added on
    host (they enter linearly).

Layout per core (SPMD, same program all 8 cores):
  inputs : xt [1024, NT] bf16  (cols = [512 shared | C routed])
           w1e [1024,4096] bf16, w2e [4096,1024] bf16  (expert c)
           ws1 [1024,4096] bf16, ws2 [4096,1024] bf16  (shared, replicated)
           b1r/bs1r [128,32] fp32 (b1 reshaped (f=ft*128+p) -> [p, ft])
  output : yt [1024, NT] fp32
"""

import numpy as np
import ml_dtypes

import concourse.bass as bass
import concourse.bacc as bacc
from concourse import mybir
from concourse.tile import TileContext
from concourse.bass_utils import run_bass_kernel_spmd

D = 1024
FF = 4096
E = 8
TOPK = 2
B, L = 4, 1024
T = B * L
NCORES = 8
SHARED = T // NCORES  # shared-expert tokens per core
P = 128
DT = D // P    # 8 k-tiles for D
FT = FF // P   # 32 tiles for FF

_BF16 = mybir.dt.bfloat16
_F32 = mybir.dt.float32

_program_cache: dict[tuple, tuple] = {}

# test harness hooks: extra kwargs for run_bass_kernel_spmd (e.g. trace=True)
# and the last BassKernelResults for profiling. Unused in normal grading runs.
TRACE_KWARGS: dict = {}
last_results = None


def _chunk_plan(C: int) -> list[int]:
    """Split C routed columns into <=512-wide chunks, sizes multiple of 8."""
    n = -(-C // 512)
    base = -(-C // n)
    base = -(-base // 8) * 8
    sizes = []
    left = C
    for _ in range(n - 1):
        sizes.append(base)
        left -= base
    sizes.append(left)
    assert all(0 < s <= 512 for s in sizes) and sum(sizes) == C
    return sizes


def _build_program(C: int):
    """One SPMD Bass program: shared FFN (512 cols) + expert FFN (C cols)."""
    NT = SHARED + C
    nc = bacc.Bacc()

    xt = nc.dram_tensor("xt", [D, NT], _BF16, kind="ExternalInput")
    w1e = nc.dram_tensor("w1e", [D, FF], _BF16, kind="ExternalInput")
    w2e = nc.dram_tensor("w2e", [FF, D], _BF16, kind="ExternalInput")
    ws1 = nc.dram_tensor("ws1", [D, FF], _BF16, kind="ExternalInput")
    ws2 = nc.dram_tensor("ws2", [FF, D], _BF16, kind="ExternalInput")
    b1r = nc.dram_tensor("b1r", [P, FT], _F32, kind="ExternalInput")
    bs1r = nc.dram_tensor("bs1r", [P, FT], _F32, kind="ExternalInput")
    yt = nc.dram_tensor("yt", [D, NT], _F32, kind="ExternalOutput")

    # chunks: (weights_key, col_offset, width); shared phase first so the
    # expert weights can stream in (reusing the same SBUF slots) while the
    # shared phase computes.
    chunks = [("s", 0, SHARED)]
    off = SHARED
    for w in _chunk_plan(C):
        chunks.append(("e", off, w))
        off += w

    with TileContext(nc) as tc:
        with (
            tc.tile_pool(name="wpool", bufs=1) as wpool,
            tc.tile_pool(name="xpool", bufs=1) as xpool,
            tc.tile_pool(name="hpool", bufs=34) as hpool,
            tc.tile_pool(name="ypool", bufs=8) as ypool,
            tc.tile_pool(name="bpool", bufs=1) as bpool,
            tc.tile_pool(name="psum", bufs=4, space="PSUM") as psum,
        ):
            b1t = bpool.tile([P, FT], _F32, tag="b1", name="b1t")
            nc.sync.dma_start(b1t, b1r[:, :])
            bs1t = bpool.tile([P, FT], _F32, tag="bs1", name="bs1t")
            nc.sync.dma_start(bs1t, bs1r[:, :])

            # W1 is stored as 8x8 [128 (d), 512 (f)] tiles (8 f-groups) so
            # the first matmul group only waits for ~1MB of DMA, not the
            # whole 8MB tensor; g-major issue order puts group 0 at the
            # DMA-queue fronts.
            FG = 8
            FGW = FF // FG  # 512 weight columns per f-group

            def _dma(dst, src):
                return nc.sync.dma_start(dst, src)

            def load_w1(src1, pfx, groups):
                t1 = {}
                for g in groups:
                    for d in range(DT):
                        t = wpool.tile([P, FGW], _BF16, tag=f"w1_{d}_{g}",
                                       name=f"{pfx}w1_{d}_{g}")
                        _dma(t, src1[d * P:(d + 1) * P, g * FGW:(g + 1) * FGW])
                        t1[(d, g)] = t
                return t1

            def load_w2(src2, pfx):
                t2 = []
                for f in range(FT):
                    t = wpool.tile([P, D], _BF16, tag=f"w2_{f}", name=f"{pfx}w2_{f}")
                    _dma(t, src2[f * P:(f + 1) * P, :])
                    t2.append(t)
                return t2

            def load_x(off, N, pfx):
                xts = []
                for d in range(DT):
                    t = xpool.tile([P, 512], _BF16, tag=f"x_{d}", name=f"{pfx}x_{d}")
                    t = t[:, :N]
                    _dma(t, xt[d * P:(d + 1) * P, off:off + N])
                    xts.append(t)
                return xts

            # DMA issue order: Ws1 group 0 + chunk-0 X (critical), then bulk.
            ws1_t = load_w1(ws1, "s_", [0])
            x0 = load_x(chunks[0][1], chunks[0][2], "c0_")
            ws1_t.update(load_w1(ws1, "s_", range(1, FG)))
            ws2_t = load_w2(ws2, "s_")
            we1_t = we2_t = None

            for ci, (kind, off, N) in enumerate(chunks):
                if kind == "s":
                    w1t, w2t, bt = ws1_t, ws2_t, bs1t
                else:
                    if we1_t is None:
                        we1_t = load_w1(w1e, "e_", range(FG))
                        we2_t = load_w2(w2e, "e_")
                    w1t, w2t, bt = we1_t, we2_t, b1t

                xts = x0 if ci == 0 else load_x(off, N, f"c{ci}_")

                hts = []
                for f in range(FT):
                    ph = psum.tile([P, 512], _F32, tag="ph", name="ph")[:, :N]
                    g, fi = divmod(f, FT // FG)
                    for d in range(DT):
                        nc.tensor.matmul(
                            ph,
                            lhsT=w1t[(d, g)][:, fi * P:(fi + 1) * P],
                            rhs=xts[d],
                            start=(d == 0),
                            stop=(d == DT - 1),
                        )
                    ht = hpool.tile([P, 512], _BF16, tag="h", name="h")[:, :N]
                    nc.scalar.activation(
                        ht, ph, mybir.ActivationFunctionType.Gelu,
                        bias=bt[:, f:f + 1],
                    )
                    hts.append(ht)

                for d in range(DT):
                    py = psum.tile([P, 512], _F32, tag="py", name="py")[:, :N]
                    for f in range(FT):
                        nc.tensor.matmul(
                            py,
                            lhsT=w2t[f][:, d * P:(d + 1) * P],
                            rhs=hts[f],
                            start=(f == 0),
                            stop=(f == FT - 1),
                        )
                    yo = ypool.tile([P, 512], _F32, tag="y", name="y")[:, :N]
                    nc.vector.tensor_copy(yo, py)
                    nc.sync.dma_start(yt[d * P:(d + 1) * P, off:off + N], yo)

    nc.finalize()
    return nc


def _get_program(C: int):
    key = (C,)
    if key not in _program_cache:
        _program_cache[key] = _build_program(C)
    return _program_cache[key]


def _route(xf: np.ndarray, W_gate: np.ndarray):
    """Replicate the reference gate in float64 (selection margins are ~1e-5,
    far above fp32 rounding, so the top-2 sets match the fp32 reference)."""
    logits = xf.astype(np.float64) @ W_gate.astype(np.float64)
    m = logits.max(axis=-1, keepdims=True)
    p = np.exp(logits - m)
    p /= p.sum(axis=-1, keepdims=True)
    top_i = np.argsort(-p, axis=-1, kind="stable")[:, :TOPK]
    top_v = np.take_along_axis(p, top_i, axis=-1)
    top_v = top_v / top_v.sum(axis=-1, keepdims=True)
    return top_i, top_v.astype(np.float32)


def kernel(x, W_gate, W1, b1, W2, b2, Ws1, bs1, Ws2, bs2):
    x = np.asarray(x, np.float32)
    xf = x.reshape(T, D)
    top_i, top_v = _route(xf, np.asarray(W_gate, np.float32))

    # per-expert token lists
    idx = [np.nonzero((top_i == e).any(axis=1))[0] for e in range(E)]
    wgt = []
    for e in range(E):
        sel = top_i[idx[e]] == e  # [cnt, K] exactly one True per row
        wgt.append(top_v[idx[e]][sel].astype(np.float32))
    counts = np.array([len(i) for i in idx])
    C = int(-(-counts.max() // 8) * 8)
    NT = SHARED + C

    xbf = xf.astype(ml_dtypes.bfloat16)
    W1 = np.asarray(W1); W2 = np.asarray(W2)
    ws1_b = np.ascontiguousarray(np.asarray(Ws1, np.float32).astype(ml_dtypes.bfloat16))
    ws2_b = np.ascontiguousarray(np.asarray(Ws2, np.float32).astype(ml_dtypes.bfloat16))
    bs1r = np.ascontiguousarray(
        np.asarray(bs1, np.float32).reshape(FT, P).T)

    in_maps = []
    for c in range(E):
        pad_idx = np.zeros(C, np.int64)
        pad_idx[:counts[c]] = idx[c]
        xcols = np.concatenate([xbf[c * SHARED:(c + 1) * SHARED], xbf[pad_idx]], axis=0)
        in_maps.append({
            "xt": np.ascontiguousarray(xcols.T),
            "w1e": np.ascontiguousarray(np.asarray(W1[c], np.float32).astype(ml_dtypes.bfloat16)),
            "w2e": np.ascontiguousarray(np.asarray(W2[c], np.float32).astype(ml_dtypes.bfloat16)),
            "ws1": ws1_b,
            "ws2": ws2_b,
            "b1r": np.ascontiguousarray(np.asarray(b1[c], np.float32).reshape(FT, P).T),
            "bs1r": bs1r,
        })

    nc = _get_program(C)
    global last_results
    last_results = run_bass_kernel_spmd(
        nc, in_maps, list(range(NCORES)), **TRACE_KWARGS)
    res = last_results.results

    out = np.zeros((T, D), np.float32)
    for c in range(E):
        y = np.asarray(res[c]["yt"], np.float32)
        out[c * SHARED:(c + 1) * SHARED] += y[:, :SHARED].T
        cnt = counts[c]
        out[idx[c]] += wgt[c][:, None] * y[:, SHARED:SHARED + cnt].T

    # biases enter linearly; add on host (zeros in this problem's inputs)
    b2 = np.asarray(b2, np.float32)
    bs2 = np.asarray(bs2, np.float32)
    combine = np.zeros((T, E), np.float32)
    np.put_along_axis(combine, top_i, top_v, axis=1)
    out += combine @ b2 + bs2

    return out.reshape(B, L, D)
